# revision 7
# baseline (speedup 1.0000x reference)
"""Trainium2 Bass kernel for nn_NeuralEmbeddingLayer.

Reference computation (B=32, T=512, N=1024):
  patches  = patchify(spikes)                       [B, 1024, 512]
  x1       = patches @ W_embed.T + b_embed          [B, 1024, 1024]
  x        = concat([cls, x1], axis=1)              [B, 1025, 1024]
  y        = gelu(x) * 32 @ W_proj.T + b_proj       [B, 1025, 1024]
  out      = y + pos_table                          (+ mask, stamp aux outputs)

Sharding: data-parallel over batch, 4 batches per core on 8 cores.
Weights replicated. The CLS row (row 0 of every batch) is input-independent
(one 1024-vector through the MLP) and is computed on host; the device
computes the 1024 patch rows per batch.

Per-core device pipeline (all matmuls in fp32r at 1 row/cycle):
  DMA patchified load -> PE transpose (A -> A^T, contraction dim on
  partitions) -> matmul1 (x1^T = W_embT.T @ A^T) -> ScalarE gelu(+b_embed
  per-partition bias) -> matmul2 (y = g^T.T @ (32*W_proj).T) -> VectorE
  (+ pos_table + b_proj) -> DMA out.
"""

import math

import numpy as np

import concourse.bass as bass
import concourse.mybir as mybir
import concourse.tile as tile
from concourse import bacc
from concourse.bass_utils import run_bass_kernel_spmd
from concourse.masks import make_identity

# Problem constants
B, T, N = 32, 512, 1024
FT, FS = 16, 32
NCH = FT * FS            # 512  patch channels (contraction dim of mm1)
DIN = NCH * 2            # 1024
H = 1024
NCLS = 1
NP = (T // FT) * (N // FS)   # 1024 patches
SCALE = float(H) ** 0.5      # 32.0

NCORES = 8
BL = B // NCORES         # 4 batches per core

F32 = mybir.dt.float32
F32R = mybir.dt.float32r


def build_nc(act_func=None):
    """Build the per-core Bass program (identical on all 8 cores).

    act_func: mybir.ActivationFunctionType override (default Gelu). CoreSim
    does not implement Gelu, so sim checks pass Tanh and compare against a
    numpy model using tanh.
    """
    if act_func is None:
        act_func = mybir.ActivationFunctionType.Gelu
    nc = bacc.Bacc(
        "TRN2",
        target_bir_lowering=False,
        debug=False,
        enable_asserts=False,
        num_devices=NCORES,
    )

    spikes = nc.dram_tensor("spikes", [BL, T, N], F32, kind="ExternalInput")
    w_embT = nc.dram_tensor("w_embT", [NCH, DIN], F32R, kind="ExternalInput")
    w_projT = nc.dram_tensor("w_projT", [DIN, H], F32R, kind="ExternalInput")
    pos2 = nc.dram_tensor("pos2", [NP, H], F32, kind="ExternalInput")
    b_emb = nc.dram_tensor("b_emb", [128, DIN // 128], F32, kind="ExternalInput")
    out = nc.dram_tensor("out", [BL, NP, H], F32, kind="ExternalOutput")

    # DRAM iteration view for the patchified load.
    # t = ((hh*4+g)*4+l)*16 + ft, n = pn*32 + fs; SBUF tile A_buf holds
    # partition=(l,pn) [128 rows of A], free=(g,ft,fs) [4 m-subtiles x 512 c].
    # DMA hardware limits each access pattern to 3 dims, so the load is one
    # DMA per (g, ft): partition<-(l,pn), free<-fs.
    spikes_r = spikes.ap().rearrange(
        "b (hh g l ft) (pn fs) -> b hh g ft l pn fs",
        hh=2, g=4, l=4, ft=FT, pn=32, fs=FS,
    )

    with tile.TileContext(nc) as tc:
        with (
            tc.tile_pool(name="const", bufs=1) as cpool,
            tc.tile_pool(name="abuf", bufs=2) as apool,
            tc.tile_pool(name="atbuf", bufs=2) as atpool,
            tc.tile_pool(name="gtbuf", bufs=2) as gtpool,
            tc.tile_pool(name="outbuf", bufs=3) as opool,
            tc.tile_pool(name="tp_ps", bufs=2, space="PSUM") as tp_ps,
            tc.tile_pool(name="mm1_ps", bufs=2, space="PSUM") as mm1_ps,
            tc.tile_pool(name="mm2_ps", bufs=2, space="PSUM") as mm2_ps,
        ):
            # Resident constants
            w_embT_s = cpool.tile([128, 4, DIN], F32R)     # [c%128, cc, d]
            w_projT_s = cpool.tile([128, 8, H], F32R)      # [d%128, dc, h]
            pos2_s = cpool.tile([128, 8, H], F32)         # [p%128, pg, h]
            b_emb_s = cpool.tile([128, 8], F32)           # [d%128, dc]
            ident = cpool.tile([128, 128], F32)

            nc.sync.dma_start(
                w_embT_s[:], w_embT.ap().rearrange("(cc p) d -> p cc d", p=128)
            )
            nc.sync.dma_start(
                w_projT_s[:], w_projT.ap().rearrange("(dc p) h -> p dc h", p=128)
            )
            nc.sync.dma_start(
                pos2_s[:], pos2.ap().rearrange("(pg p) h -> p pg h", p=128)
            )
            nc.sync.dma_start(b_emb_s[:], b_emb.ap())
            make_identity(nc, ident[:])

            for b in range(BL):
                for hh in range(2):  # half-batch: 512 patch rows each
                    # ---- load (patchify gather) ----
                    A_buf = apool.tile([128, 4, FT, FS], F32)
                    for g in range(4):
                        for ft in range(FT):
                            nc.sync.dma_start(
                                A_buf[:, g, ft, :], spikes_r[b, hh, g, ft]
                            )

                    # ---- transpose A -> A^T (c on partitions) ----
                    A_T = atpool.tile([128, 4, 512], F32R)  # [c%128, cc, m]
                    for g in range(4):
                        for cc in range(4):
                            tp = tp_ps.tile([128, 128], F32)
                            blk = A_buf[:, g, 4 * cc:4 * cc + 4, :]
                            nc.tensor.transpose(tp[:], blk, ident[:])
                            nc.vector.tensor_copy(
                                A_T[:, cc, g * 128:(g + 1) * 128], tp[:]
                            )

                    # ---- mm1 + gelu: g^T[d, m] = gelu(W_embT.T @ A^T + b) ----
                    g_T = gtpool.tile([128, 8, 512], F32R)  # [d%128, dc, m]
                    for dc in range(8):
                        x1t = mm1_ps.tile([128, 512], F32)
                        for cc in range(4):
                            nc.tensor.matmul(
                                x1t[:],
                                w_embT_s[:, cc, dc * 128:(dc + 1) * 128],
                                A_T[:, cc, :],
                                start=(cc == 0),
                                stop=(cc == 3),
                            )
                        nc.scalar.activation(
                            g_T[:, dc, :],
                            x1t[:],
                            act_func,
                            bias=b_emb_s[:, dc:dc + 1],
                        )

                    # ---- mm2 + pos add + store ----
                    for g in range(4):
                        out_t = opool.tile([128, H], F32)
                        for hc in range(2):
                            y = mm2_ps.tile([128, 512], F32)
                            for dc in range(8):
                                nc.tensor.matmul(
                                    y[:],
                                    g_T[:, dc, g * 128:(g + 1) * 128],
                                    w_projT_s[:, dc, hc * 512:hc * 512 + 512],
                                    start=(dc == 0),
                                    stop=(dc == 7),
                                )
                            nc.vector.tensor_add(
                                out_t[:, hc * 512:hc * 512 + 512],
                                y[:],
                                pos2_s[:, hh * 4 + g, hc * 512:hc * 512 + 512],
                            )
                        p0 = hh * 512 + g * 128
                        nc.scalar.dma_start(out.ap()[b, p0:p0 + 128, :], out_t[:])

    nc.compile()
    return nc


def _host_prep(W_embed, b_embed, cls_tokens, W_proj, b_proj, pos_table):
    """Precompute device weight layouts + the (input-independent) CLS row."""
    w_embT = np.ascontiguousarray(W_embed.T).astype(np.float32)          # [c, d]
    w_projT = np.ascontiguousarray((W_proj * SCALE).T).astype(np.float32)  # [d, h]
    pos2 = (pos_table[NCLS:] + b_proj[None, :]).astype(np.float32)       # [NP, H]
    b_emb = np.ascontiguousarray(b_embed.reshape(8, 128).T).astype(np.float32)

    # CLS row: gelu(cls)*SCALE @ W_proj.T + b_proj + pos_table[0]
    v = cls_tokens.reshape(-1)[:DIN].astype(np.float64)
    erf = np.vectorize(math.erf)
    gelu_v = v * 0.5 * (1.0 + erf(v / math.sqrt(2.0)))
    cls_row = (gelu_v * SCALE) @ W_proj.astype(np.float64).T + b_proj + pos_table[0]
    cls_row = cls_row.astype(np.float32)
    return w_embT, w_projT, pos2, b_emb, cls_row


_NC_CACHE = {}


def kernel(spikes, W_embed, b_embed, cls_tokens, W_proj, b_proj, pos_table):
    spikes = np.asarray(spikes, dtype=np.float32)
    W_embed = np.asarray(W_embed, dtype=np.float32)
    b_embed = np.asarray(b_embed, dtype=np.float32)
    cls_tokens = np.asarray(cls_tokens, dtype=np.float32)
    W_proj = np.asarray(W_proj, dtype=np.float32)
    b_proj = np.asarray(b_proj, dtype=np.float32)
    pos_table = np.asarray(pos_table, dtype=np.float32)

    w_embT, w_projT, pos2, b_emb, cls_row = _host_prep(
        W_embed, b_embed, cls_tokens, W_proj, b_proj, pos_table
    )

    if "nc" not in _NC_CACHE:
        _NC_CACHE["nc"] = build_nc()
    nc = _NC_CACHE["nc"]

    in_maps = []
    for c in range(NCORES):
        in_maps.append({
            "spikes": np.ascontiguousarray(spikes[c * BL:(c + 1) * BL]),
            "w_embT": w_embT,
            "w_projT": w_projT,
            "pos2": pos2,
            "b_emb": b_emb,
        })

    res = run_bass_kernel_spmd(nc, in_maps, list(range(NCORES)))

    x = np.empty((B, NP + NCLS, H), dtype=np.float32)
    x[:, 0, :] = cls_row[None, :]
    for c in range(NCORES):
        x[c * BL:(c + 1) * BL, NCLS:, :] = res.results[c]["out"]

    mask = np.ones((B, NP + NCLS), dtype=np.int32)
    stamp = np.broadcast_to(
        np.arange(NP + NCLS, dtype=np.int32)[None, :], (B, NP + NCLS)
    ).copy()
    return x, mask, stamp
